# revision 24
# baseline (speedup 1.0000x reference)
"""Bahdanau additive-attention kernel for Trainium2 (8 NeuronCores, SPMD).

Problem: values [B=8, Te=512, De=256], query [B=8, Td=128, Dq=256],
W_a [256,128], U_a [256,128], V_a [128,1].
  Wh = values @ W_a                    [B,Te,A]
  Us = query @ U_a                     [B,Td,A]
  scores[b,q,e] = sum_a V[a] * tanh(Wh[b,e,a] + Us[b,q,a])
  e = softmax(scores, axis=e)          [B,Td,Te]
  c = e @ values                       [B,Td,De]
returns (c, e).

Sharding: data-parallel over batch B across the 8 cores (one element each),
weights replicated. No collectives.

Per-core layout: A=128 on partitions.  WhT [a,e], UsT [a,q] from PE matmuls;
X[a, (q,e)] = WhT + UsT[:,q] via per-q DVE tensor_scalar_add (bf16);
tanh on ScalarE in big chunks (the critical path ~55us);
scores via M=1 col-tiled PE matmuls with V as stationary (4 concurrent);
PSUM -> SBUF (DVE full-tile copy) -> partition remap via SBUF->SBUF DMA;
exp with fused accum_out sum; context via transposed-E matmuls.
"""

import numpy as np

B, TE, TD, DE, DQ, A = 8, 512, 128, 256, 256, 128
N_CORES = 8
QCHUNK = 16  # q's per tanh chunk (4 score groups of 4)

_CACHE = {}


def _body(ctx, tc, values, query, W_a, U_a, V_a, c_out, e_out):
    from concourse import mybir, masks

    nc = tc.nc
    f32 = mybir.dt.float32
    bf16 = mybir.dt.bfloat16
    Act = mybir.ActivationFunctionType

    const = ctx.enter_context(tc.tile_pool(name="const", bufs=1))
    xpool = ctx.enter_context(tc.tile_pool(name="x", bufs=6))
    stage_pool = ctx.enter_context(tc.tile_pool(name="stage", bufs=3))
    ps_scores = ctx.enter_context(tc.tile_pool(name="ps_scores", bufs=2, space="PSUM"))
    ps_misc = ctx.enter_context(tc.tile_pool(name="ps_misc", bufs=3, space="PSUM"))

    # ---------- load + cast inputs ----------
    ident = const.tile([128, 128], bf16, tag="ident")
    masks.make_identity(nc, ident[:])

    v_f32 = const.tile([128, 4, DE], f32, tag="vf32")
    v_view = values.rearrange("(c p) d -> p c d", p=128)
    nc.sync.dma_start(v_f32[:, 0:2, :], v_view[:, 0:2, :])
    nc.sync.dma_start(v_f32[:, 2:4, :], v_view[:, 2:4, :])
    v_bf = const.tile([128, 4, DE], bf16, tag="vbf")
    nc.vector.tensor_copy(v_bf[:], v_f32[:])

    # warm the ACT tanh/exp table set early (overlaps the input DMAs)
    warm = const.tile([128, 1], f32, tag="warm")
    nc.gpsimd.memset(warm[:], 0.0)
    nc.scalar.activation(warm[:], warm[:], Act.Tanh)

    q_f32 = const.tile([128, DQ], f32, tag="qf32")
    nc.sync.dma_start(q_f32[:], query[:, :])

    w_f32 = const.tile([128, 2, A], f32, tag="wf32")
    nc.sync.dma_start(w_f32[:], W_a.rearrange("(k p) a -> p k a", p=128))
    w_bf = const.tile([128, 2, A], bf16, tag="wbf")
    nc.vector.tensor_copy(w_bf[:], w_f32[:])

    u_f32 = const.tile([128, 2, A], f32, tag="uf32")
    nc.sync.dma_start(u_f32[:], U_a.rearrange("(k p) a -> p k a", p=128))
    u_bf = const.tile([128, 2, A], bf16, tag="ubf")
    nc.vector.tensor_copy(u_bf[:], u_f32[:])

    va_f32 = const.tile([128, 1], f32, tag="vaf32")
    nc.sync.dma_start(va_f32[:], V_a[:, :])
    # V replicated across 32 columns: M=32 score matmuls fill a whole PSUM bank
    v32_bf = const.tile([128, 32], bf16, tag="v32bf")
    nc.vector.tensor_copy(v32_bf[:], va_f32[:].broadcast_to([128, 32]))

    # ---------- transposes: valT [d-half, e], qT [dq-half, q] ----------
    # transpose straight from f32 (cast to bf16 in the PSUM->SBUF copy) so
    # the Wh chain doesn't wait on the full v_bf cast
    ident_f32 = const.tile([128, 128], f32, tag="ident_f32")
    masks.make_identity(nc, ident_f32[:])
    valT = [
        const.tile([128, TE], bf16, tag=f"valT{h}", name=f"valT{h}")
        for h in range(2)
    ]
    for h in range(2):
        for c in range(4):
            pst = ps_misc.tile([128, 128], f32, tag="misc")
            nc.tensor.transpose(pst[:], v_f32[:, c, h * 128:(h + 1) * 128], ident_f32[:])
            nc.vector.tensor_copy(valT[h][:, c * 128:(c + 1) * 128], pst[:])
    qT = const.tile([128, 2, TD], bf16, tag="qT")
    for h in range(2):
        pst = ps_misc.tile([128, 128], f32, tag="misc")
        nc.tensor.transpose(pst[:], q_f32[:, h * 128:(h + 1) * 128], ident_f32[:])
        nc.vector.tensor_copy(qT[:, h, :], pst[:])

    # ---------- WhT [a, e], UsT [a, q] ----------
    # two e-halves so chunk-0 adds can start after half the transpose chain
    WhT = const.tile([128, TE], bf16, tag="WhT")
    for eh in range(2):
        ps_wh = ps_misc.tile([128, TE // 2], f32, tag="misc", name="ps_wh")
        sl = slice(eh * (TE // 2), (eh + 1) * (TE // 2))
        nc.tensor.matmul(ps_wh[:], w_bf[:, 0, :], valT[0][:, sl], start=True, stop=False)
        nc.tensor.matmul(ps_wh[:], w_bf[:, 1, :], valT[1][:, sl], start=False, stop=True)
        nc.vector.tensor_copy(WhT[:, sl], ps_wh[:])

    ps_us = ps_misc.tile([128, TD], f32, tag="misc")
    nc.tensor.matmul(ps_us[:], u_bf[:, 0, :], qT[:, 0, :], start=True, stop=False)
    nc.tensor.matmul(ps_us[:], u_bf[:, 1, :], qT[:, 1, :], start=False, stop=True)
    UsT = const.tile([128, TD], f32, tag="UsT")
    nc.vector.tensor_copy(UsT[:], ps_us[:])

    # ---------- main loop: broadcast-add, tanh, V-reduce, evac ----------
    # Emission is software-pipelined: chunk ch's adds+tanh are emitted before
    # chunk ch-1's score matmuls/evac, so the DVE stream never puts evac
    # copies between an adds batch and its tanh. The last chunks are small to
    # shorten the post-tanh drain.
    scores = const.tile([128, TE], f32, tag="scores")
    sizes = [4, 12] + [16] * 6 + [12, 4]
    assert sum(sizes) == TD
    chunk_q0 = [sum(sizes[:i]) for i in range(len(sizes))]
    xs = []  # (X tile, q0, nq) awaiting score processing

    def emit_scores(X, q0, nq):
        for gl in range(nq // 4):
            g = (q0 + 4 * gl) // 4  # global group; covers q = 4g..4g+3
            if g % 2 == 0:
                ps2 = ps_scores.tile([128, 2 * TE], f32, tag="sc", name="ps2")
                stg2 = stage_pool.tile([128, 2 * TE], f32, tag="stg", name="stg2")
                emit_scores.ps2, emit_scores.stg2 = ps2, stg2
            else:
                ps2, stg2 = emit_scores.ps2, emit_scores.stg2
            half = (g % 2) * TE
            for j2 in range(4):
                col = gl * 4 + j2
                nc.tensor.matmul(
                    ps2[32 * j2:32 * j2 + 32, half:half + TE],
                    v32_bf[:],
                    X[:, col * TE:(col + 1) * TE],
                    start=True,
                    stop=True,
                    tile_position=(0, 32 * j2),
                )
            nc.vector.tensor_copy(
                stg2[:, half:half + TE], ps2[:, half:half + TE]
            )
            # rows {0,32,64,96} hold q=4g..4g+3; remap via SBUF->SBUF DMA
            stg_v = stg2[:, half:half + TE].rearrange(
                "(j r) e -> j r e", j=4
            )[:, 0, :]
            nc.sync.dma_start(scores[4 * g:4 * g + 4, :], stg_v)

    for ch in range(len(sizes)):
        nq = sizes[ch]
        X = xpool.tile([128, nq * TE], bf16, tag="X", name="X")
        if ch == 0:
            # chunk 0: split by e-half so adds start after WhT half 0
            for eh in range(2):
                sl = slice(eh * (TE // 2), (eh + 1) * (TE // 2))
                for j in range(nq):
                    q = chunk_q0[ch] + j
                    nc.vector.tensor_scalar_add(
                        X[:, j * TE + sl.start:j * TE + sl.stop],
                        WhT[:, sl],
                        UsT[:, q:q + 1],
                    )
        else:
            for j in range(nq):
                q = chunk_q0[ch] + j
                nc.vector.tensor_scalar_add(
                    X[:, j * TE:(j + 1) * TE], WhT[:], UsT[:, q:q + 1]
                )
        nc.scalar.activation(X[:], X[:], Act.Tanh)
        xs.append((X, chunk_q0[ch], nq))
        if ch >= 1:
            emit_scores(*xs[ch - 1])
    emit_scores(*xs[-1])

    # ---------- softmax over e (scores are small: skip max-subtract) ----------
    E_bf = const.tile([128, TE], bf16, tag="E")
    sumE = const.tile([128, 1], f32, tag="sumE")
    nc.scalar.activation(E_bf[:], scores[:], Act.Exp, accum_out=sumE[:])
    r = const.tile([128, 1], f32, tag="r")
    nc.vector.reciprocal(r[:], sumE[:])
    e_sb = const.tile([128, TE], f32, tag="e_sb")
    nc.vector.tensor_scalar_mul(e_sb[:], E_bf[:], r[:])
    nc.sync.dma_start(e_out[:, :], e_sb[:])

    # ---------- context: c = (E @ values) * r ----------
    eT = const.tile([128, 4, 128], bf16, tag="eT")
    for k in range(4):
        pst = ps_misc.tile([128, 128], bf16, tag="misc")
        nc.tensor.transpose(pst[:], E_bf[:, k * 128:(k + 1) * 128], ident[:])
        nc.vector.tensor_copy(eT[:, k, :], pst[:])
    ps_c = ps_misc.tile([128, DE], f32, tag="misc")
    for k in range(4):
        nc.tensor.matmul(
            ps_c[:], eT[:, k, :], v_bf[:, k, :], start=(k == 0), stop=(k == 3)
        )
    c_sb = const.tile([128, DE], f32, tag="c_sb")
    nc.vector.tensor_scalar_mul(c_sb[:], ps_c[:], r[:])
    nc.sync.dma_start(c_out[:, :], c_sb[:])


def build_nc():
    """Build + compile the single-core Bass graph (same graph on all 8 cores)."""
    if "nc" in _CACHE:
        return _CACHE["nc"]
    from contextlib import ExitStack

    from concourse import bacc, mybir, tile

    f32 = mybir.dt.float32
    nc = bacc.Bacc("TRN2", target_bir_lowering=False, debug=False)
    values = nc.dram_tensor("values", [TE, DE], f32, kind="ExternalInput").ap()
    query = nc.dram_tensor("query", [TD, DQ], f32, kind="ExternalInput").ap()
    W_a = nc.dram_tensor("W_a", [DE, A], f32, kind="ExternalInput").ap()
    U_a = nc.dram_tensor("U_a", [DQ, A], f32, kind="ExternalInput").ap()
    V_a = nc.dram_tensor("V_a", [A, 1], f32, kind="ExternalInput").ap()
    c_out = nc.dram_tensor("c_out", [TD, DE], f32, kind="ExternalOutput").ap()
    e_out = nc.dram_tensor("e_out", [TD, TE], f32, kind="ExternalOutput").ap()

    with tile.TileContext(nc) as tc, ExitStack() as ctx:
        _body(ctx, tc, values, query, W_a, U_a, V_a, c_out, e_out)
    nc.compile()
    _CACHE["nc"] = nc
    return nc


def _in_maps(inputs):
    values = np.ascontiguousarray(inputs["values"], dtype=np.float32)
    query = np.ascontiguousarray(inputs["query"], dtype=np.float32)
    W_a = np.ascontiguousarray(inputs["W_a"], dtype=np.float32)
    U_a = np.ascontiguousarray(inputs["U_a"], dtype=np.float32)
    V_a = np.ascontiguousarray(inputs["V_a"], dtype=np.float32)
    return [
        {
            "values": values[b],
            "query": query[b],
            "W_a": W_a,
            "U_a": U_a,
            "V_a": V_a,
        }
        for b in range(B)
    ]


def run(inputs, trace=False):
    """Run on the 8 NeuronCores; returns (BassKernelResults, c, e)."""
    from concourse.bass_utils import run_bass_kernel_spmd

    nc = build_nc()
    res = run_bass_kernel_spmd(
        nc, _in_maps(inputs), core_ids=list(range(N_CORES)), trace=trace
    )
    c = np.stack([res.results[b]["c_out"] for b in range(B)])
    e = np.stack([res.results[b]["e_out"] for b in range(B)])
    return res, c, e


def kernel(**inputs):
    _, c, e = run(inputs, trace=False)
    return (c, e)
